# revision 1
# baseline (speedup 1.0000x reference)
"""Mixtral sparse-MoE block on 8 Trainium2 NeuronCores (expert parallel).

Strategy (matches the module's shard_map): expert weights are sharded along E
(one expert per core), hidden_states replicated. Each core:
  - computes router logits for all tokens (gate matmul, fp32r),
  - derives its expert's renormalized top-2 combine weight per token on-chip
    (top2-softmax == sigmoid of the top-2 logit difference, done via tanh so
    the whole kernel stays in the `silu` ACT table set — zero table swaps),
  - runs its expert's up/gate/down matmuls densely over all tokens (fp32r),
  - scales its expert output by the combine column,
  - psum-combines across cores with a per-chunk ReduceScatter.
Host side only shards/unshards (transpose + concat).

Layouts: everything on-chip is feature-major ("xT" = x transposed) so all
three expert matmuls keep weights as the stationary operand and tokens as the
moving free dim (N=512), with zero on-chip transposes of activations.
"""

import numpy as np

import concourse.bass as bass
import concourse.mybir as mybir
import concourse.tile as tile
from concourse import bacc
from concourse.bass_utils import run_bass_kernel_spmd
from concourse.masks import make_identity

# Problem shape (hardcoded per contract).
B, S, H, F, E = 2, 2048, 1024, 2048, 8
T = B * S                    # 4096 tokens
N_CORES = 8
HC = H // 128                # 8 h-chunks
FC = F // 128                # 16 f-chunks
NTQ = 4                      # token quarters
TOK = T // NTQ               # 1024 tokens per quarter
TN = TOK // 512              # 512-token subchunks per quarter
BIG = 1e30

f32 = mybir.dt.float32
f32r = mybir.dt.float32r
AF = mybir.ActivationFunctionType
ALU = mybir.AluOpType
AX = mybir.AxisListType


def build(use_rs=True, stub_router=False):
    nc = bacc.Bacc("TRN2", target_bir_lowering=False, debug=False,
                   num_devices=N_CORES)

    xT = nc.dram_tensor("xT", [H, T], f32r, kind="ExternalInput").ap()
    gw = nc.dram_tensor("gw", [H, E], f32r, kind="ExternalInput").ap()
    wu = nc.dram_tensor("wu", [H, F], f32r, kind="ExternalInput").ap()
    wg = nc.dram_tensor("wg", [H, F], f32r, kind="ExternalInput").ap()
    wd = nc.dram_tensor("wd", [F, H], f32r, kind="ExternalInput").ap()
    sel = nc.dram_tensor("sel", [128, E], f32, kind="ExternalInput").ap()
    if use_rs:
        yp = nc.dram_tensor("yp", [128, T], f32, kind="ExternalOutput").ap()
    else:
        yp = nc.dram_tensor("yp", [H, T], f32, kind="ExternalOutput").ap()

    # DRAM views with the 128-partition dim pulled out front.
    xT_v = xT.rearrange("(hc p) t -> p hc t", p=128)     # [128, 8, 4096]
    gw_v = gw.rearrange("(hc p) e -> p hc e", p=128)     # [128, 8, 8]
    wu_v = wu.rearrange("(hc p) f -> p hc f", p=128)     # [128, 8, 2048]
    wg_v = wg.rearrange("(hc p) f -> p hc f", p=128)
    wd_v = wd.rearrange("(fc p) h -> p fc h", p=128)     # [128, 16, 1024]

    with tile.TileContext(nc) as tc:
        with (
            tc.tile_pool(name="const", bufs=1) as cpool,
            tc.tile_pool(name="xq", bufs=2) as xqpool,
            tc.tile_pool(name="w", bufs=2) as wpool,
            tc.tile_pool(name="inner", bufs=1) as ipool,
            tc.tile_pool(name="work", bufs=3) as spool,
            tc.tile_pool(name="cbc", bufs=2) as cbcpool,
            tc.tile_pool(name="router", bufs=2) as rpool,
            tc.tile_pool(name="psum", bufs=2, space="PSUM") as psum,
            tc.tile_pool(name="dram", bufs=2, space="DRAM") as dram,
        ):
            # Constants
            id8 = cpool.tile([E, E], f32)
            make_identity(nc, id8[:])
            id128 = cpool.tile([128, 128], f32)
            make_identity(nc, id128[:])
            ones1 = cpool.tile([1, 128], f32)
            nc.gpsimd.memset(ones1[:], 1.0)
            sel_sb = cpool.tile([128, E], f32)
            nc.sync.dma_start(sel_sb[:], sel[:])
            gw_sb = cpool.tile([128, HC, E], f32r)
            nc.sync.dma_start(gw_sb[:], gw_v[:])

            def load_xq(tq):
                tsl = slice(tq * TOK, (tq + 1) * TOK)
                t = xqpool.tile([128, HC, TOK], f32r, tag="xq")
                for hc in range(HC):
                    nc.sync.dma_start(t[:, hc, :], xT_v[:, hc, tsl])
                return t

            def load_w(fc):
                fs = slice(fc * 128, (fc + 1) * 128)
                wu_t = wpool.tile([128, HC, 128], f32r, tag="wu")
                nc.sync.dma_start(wu_t[:], wu_v[:, :, fs])
                wg_t = wpool.tile([128, HC, 128], f32r, tag="wg")
                nc.sync.dma_start(wg_t[:], wg_v[:, :, fs])
                return wu_t, wg_t

            w_pre = None  # next quarter's (fc=0) up/gate weights
            xq_next = load_xq(0)
            for tq in range(NTQ):
                ts = slice(tq * TOK, (tq + 1) * TOK)
                xq = xq_next

                # ---- Router part 1: logits (PE work, up front) ----
                c_bc = []  # [128, 512] broadcast combine weight per tn
                logT_sbs = []
                if stub_router:
                    for tn in range(TN):
                        cb = cbcpool.tile([128, 512], f32, tag=f"cbc{tn}")
                        nc.gpsimd.memset(cb[:], 1.0)
                        c_bc.append(cb)
                for tn in range(TN) if not stub_router else []:
                    tns = slice(tn * 512, (tn + 1) * 512)
                    logT_ps = psum.tile([E, 512], f32, tag="up")
                    for hc in range(HC):
                        nc.tensor.matmul(logT_ps[:], gw_sb[:, hc, :],
                                         xq[:, hc, tns],
                                         start=(hc == 0), stop=(hc == HC - 1))
                    logT_sb = rpool.tile([E, 512], f32, tag="logT")
                    nc.vector.tensor_copy(logT_sb[:], logT_ps[:])
                    logT_sbs.append(logT_sb)

                # ---- Phase A: up/gate matmuls + silu -> inner ----
                inner = [[None] * TN for _ in range(FC)]
                for fc in range(FC):
                    wu_t, wg_t = w_pre if (fc == 0 and w_pre) else load_w(fc)
                    for tn in range(TN):
                        tns = slice(tn * 512, (tn + 1) * 512)
                        up_ps = psum.tile([128, 512], f32, tag="up")
                        for hc in range(HC):
                            nc.tensor.matmul(up_ps[:], wu_t[:, hc, :],
                                             xq[:, hc, tns],
                                             start=(hc == 0),
                                             stop=(hc == HC - 1))
                        gate_ps = psum.tile([128, 512], f32, tag="gate")
                        for hc in range(HC):
                            nc.tensor.matmul(gate_ps[:], wg_t[:, hc, :],
                                             xq[:, hc, tns],
                                             start=(hc == 0),
                                             stop=(hc == HC - 1))
                        sg_sb = spool.tile([128, 512], f32, tag="sg")
                        nc.scalar.activation(sg_sb[:], up_ps[:], AF.Sigmoid)
                        silu_sb = spool.tile([128, 512], f32, tag="silu")
                        nc.vector.tensor_mul(silu_sb[:], sg_sb[:], up_ps[:])
                        it = ipool.tile([128, 512], f32r, tag=f"i{fc}_{tn}")
                        nc.vector.tensor_mul(it[:], silu_sb[:], gate_ps[:])
                        inner[fc][tn] = it

                # Prefetch next quarter's activations and first up/gate
                # weights during phase C.
                if tq + 1 < NTQ:
                    xq_next = load_xq(tq + 1)
                    w_pre = load_w(0)
                else:
                    w_pre = None

                # ---- Router part 2: top-2 combine weight (vector math) ----
                for tn in range(TN) if not stub_router else []:
                    logT_sb = logT_sbs[tn]
                    c_row = rpool.tile([1, 512], f32, tag="crow")
                    for tcj in range(4):
                        cs = slice(tcj * 128, (tcj + 1) * 128)
                        tr_ps = psum.tile([128, E], f32, tag="tr")
                        nc.tensor.transpose(tr_ps[:], logT_sb[:, cs], id8[:])
                        L = rpool.tile([128, E], f32, tag="L")
                        nc.vector.tensor_copy(L[:], tr_ps[:])

                        m1 = rpool.tile([128, 1], f32, tag="m1")
                        nc.vector.reduce_max(m1[:], L[:], axis=AX.X)
                        mask1 = rpool.tile([128, E], f32, tag="mask1")
                        nc.vector.tensor_scalar(mask1[:], L[:], m1[:], None,
                                                op0=ALU.is_ge)
                        big = rpool.tile([128, E], f32, tag="big")
                        nc.vector.tensor_scalar_mul(big[:], mask1[:], BIG)
                        Lm = rpool.tile([128, E], f32, tag="Lm")
                        nc.vector.tensor_sub(Lm[:], L[:], big[:])
                        m2 = rpool.tile([128, 1], f32, tag="m2")
                        nc.vector.reduce_max(m2[:], Lm[:], axis=AX.X)
                        mask2 = rpool.tile([128, E], f32, tag="mask2")
                        nc.vector.tensor_scalar(mask2[:], L[:], m2[:], None,
                                                op0=ALU.is_ge)

                        # s1 = sigmoid(m1-m2) via tanh; s2 = 1-s1
                        d = rpool.tile([128, 1], f32, tag="d")
                        nc.vector.tensor_sub(d[:], m1[:], m2[:])
                        th = rpool.tile([128, 1], f32, tag="th")
                        nc.scalar.activation(th[:], d[:], AF.Tanh, scale=0.5)
                        s1 = rpool.tile([128, 1], f32, tag="s1")
                        nc.vector.tensor_scalar(s1[:], th[:], 0.5, 0.5,
                                                op0=ALU.mult, op1=ALU.add)
                        s2 = rpool.tile([128, 1], f32, tag="s2")
                        nc.vector.tensor_scalar(s2[:], th[:], -0.5, 0.5,
                                                op0=ALU.mult, op1=ALU.add)

                        # a = is-top1 for this expert; b = is-top2
                        scr_a = rpool.tile([128, E], f32, tag="scra")
                        nc.vector.tensor_mul(scr_a[:], mask1[:], sel_sb[:])
                        a_col = rpool.tile([128, 1], f32, tag="acol")
                        nc.vector.reduce_sum(a_col[:], scr_a[:], axis=AX.X)
                        bmask = rpool.tile([128, E], f32, tag="bmask")
                        nc.vector.tensor_sub(bmask[:], mask2[:], mask1[:])
                        scr_b = rpool.tile([128, E], f32, tag="scrb")
                        nc.vector.tensor_mul(scr_b[:], bmask[:], sel_sb[:])
                        b_col = rpool.tile([128, 1], f32, tag="bcol")
                        nc.vector.reduce_sum(b_col[:], scr_b[:], axis=AX.X)

                        pa = rpool.tile([128, 1], f32, tag="pa")
                        nc.vector.tensor_mul(pa[:], a_col[:], s1[:])
                        pb = rpool.tile([128, 1], f32, tag="pb")
                        nc.vector.tensor_mul(pb[:], b_col[:], s2[:])
                        c_col = rpool.tile([128, 1], f32, tag="ccol")
                        nc.vector.tensor_add(c_col[:], pa[:], pb[:])

                        ctr_ps = psum.tile([1, 128], f32, tag="tr")
                        nc.tensor.transpose(ctr_ps[:], c_col[:], id128[:])
                        nc.scalar.copy(c_row[:, cs], ctr_ps[:])

                    bc_ps = psum.tile([128, 512], f32, tag="y")
                    nc.tensor.matmul(bc_ps[:], ones1[:], c_row[:],
                                     start=True, stop=True)
                    cb = cbcpool.tile([128, 512], f32, tag=f"cbc{tn}")
                    nc.vector.tensor_copy(cb[:], bc_ps[:])
                    c_bc.append(cb)

                # ---- Phase C: down matmul + combine scale ----
                # ReduceScatter is row-split in two halves so the first half
                # overlaps the second half's compute, and the exposed tail is
                # a 2MB collective instead of 4MB.
                yt = dram.tile([H, TOK], f32, tag="yt")
                for hc in range(HC):
                    hs = slice(hc * 128, (hc + 1) * 128)
                    wd_t = wpool.tile([128, FC, 128], f32r, tag="wd")
                    nc.sync.dma_start(wd_t[:], wd_v[:, :, hs])
                    for tn in range(TN):
                        tns = slice(tn * 512, (tn + 1) * 512)
                        y_ps = psum.tile([128, 512], f32, tag="y")
                        for fcj in range(FC):
                            nc.tensor.matmul(y_ps[:], wd_t[:, fcj, :],
                                             inner[fcj][tn][:],
                                             start=(fcj == 0),
                                             stop=(fcj == FC - 1))
                        y_sb = spool.tile([128, 512], f32, tag="ysb")
                        nc.vector.tensor_mul(y_sb[:], y_ps[:], c_bc[tn][:])
                        nc.gpsimd.dma_start(yt[hs, tns], y_sb[:])

                # ---- psum-combine over the expert axis ----
                if use_rs:
                    rs_out = dram.tile([128, TOK], f32, tag="rs")
                    nc.gpsimd.collective_compute(
                        "ReduceScatter", ALU.add,
                        replica_groups=[list(range(N_CORES))],
                        ins=[yt.opt()], outs=[rs_out.opt()])
                    nc.gpsimd.dma_start(yp[:, ts], rs_out[:])
                else:
                    nc.gpsimd.dma_start(yp[:, ts], yt[:])

    nc.compile()
    return nc


_CACHED = None
_USE_RS = True


def _get_program():
    global _CACHED
    if _CACHED is None:
        _CACHED = build(use_rs=_USE_RS)
    return _CACHED


def kernel(hidden_states, gate_w, w_up, w_gate, w_down):
    nc = _get_program()
    x = np.asarray(hidden_states, np.float32).reshape(T, H)
    xT_np = np.ascontiguousarray(x.T)                    # [H, T]
    gw_np = np.ascontiguousarray(np.asarray(gate_w, np.float32))
    in_maps = []
    for c in range(N_CORES):
        selm = np.zeros((128, E), np.float32)
        selm[:, c] = 1.0
        in_maps.append({
            "xT": xT_np,
            "gw": gw_np,
            "wu": np.ascontiguousarray(np.asarray(w_up[c], np.float32)),
            "wg": np.ascontiguousarray(np.asarray(w_gate[c], np.float32)),
            "wd": np.ascontiguousarray(np.asarray(w_down[c], np.float32)),
            "sel": selm,
        })
    res = run_bass_kernel_spmd(nc, in_maps, list(range(N_CORES)))
    if _USE_RS:
        yT = np.concatenate([res.results[c]["yp"] for c in range(N_CORES)],
                            axis=0)
    else:
        yT = np.sum([res.results[c]["yp"] for c in range(N_CORES)], axis=0)
    return np.ascontiguousarray(yT.T).reshape(B, S, H).astype(np.float32)

